# revision 95
# baseline (speedup 1.0000x reference)
"""Trainium2 Bass kernel for a dense transformer block (MAB-style).

Reference computation (per batch b of 32, seq 512, dim 512, 8 heads):
    q = Q @ Wq.T + bq ; k = K @ Wk.T + bk ; v = V @ Wv.T + bv
    scores = (qh . kh) / sqrt(512) ; A = softmax(scores, axis=j)
    o = qh + A @ vh                       (residual on projected q)
    X = LN0(o) ; O = X + relu(X @ Wo.T + bo) ; O = LN1(O)

Sharding: pure data parallel, 4 batches per core x 8 cores (no collectives).

Device-side layout strategy (per core):
  - Q/K/V are pre-transposed on the host to [d, seq] (bf16) and arrive as
    one batched DMA per tensor, so every matmul operand already has its
    contraction dim on partitions.
  - Projections produce qT/kT [e, i] (bf16) and v natural [j, e] in fp8
    with an extra ones column, the two j-chunks of a DoubleRow k-tile pair
    stacked on dim 1: v8 [128, 2, 8, 65].
  - scoresT [j, i] per head = matmul(lhsT=khT slice, rhs=qhT slice), one
    PSUM bank per head; softmax exp on ACT with the 1/sqrt(512) scale
    folded into the activation's free affine, written straight to fp8 pt.
  - AV runs in NATURAL orientation with fp8 DoubleRow matmuls (two j-chunks
    per instruction at half a cycle per output row): out[i, (h, 64)] =
    matmul(lhsT=pt[j-pair, i-block], rhs=v8[j-pair, h, 0:64]).  A whole head
    pair fits one PSUM bank ([128, 4, 2, 64] = 2048B); softmax denominators
    come from N=1 matmuls against the v8 ones column into a shared 1-bank
    accumulator, so no transpose of the attention output is ever needed.
  - q_nat and n0T are produced by DMA-XBAR transposes (dma_start_transpose)
    into [128, chunk, 512] tiles: zero PE and zero DVE cost.
  - x0 = q_nat + o*(1/s) via per-head scalar_tensor_tensor on DVE;
    LayerNorms use bn_stats/bn_aggr with a 3-iteration Newton rsqrt on
    GPSIMD (DVE for the last batch) so ACT never leaves the exp table set.
  - MLP: matmul with WoT (g0 folded host-side); relu runs on ACT as the
    PSUM drain and the residual add on DVE at bf16 2x rate; LN1; DMA out
    as bf16 (host converts to f32).
  - Engine economy: ACT owns exp (the serial backbone) plus the MLP relu
    drains; DVE owns all projection drains, x0, and LN stats; GPSIMD owns
    Newton, n0/out normalizes, and the mid-batch output DMAs (SWDGE), so
    output transfers never head-of-line block input prefetches on the
    SP/HWDGE queue.  PSUM is exactly 8 banks: proj 2 + MLP 1 + scores 2 +
    pair-AV 2 + denominators 1, with tags split per stage so no pool ring
    makes batch b+1's projections wait on batch b's MLP.
"""

import math
from contextlib import ExitStack

import numpy as np

B, S, D = 32, 512, 512
H = 8
DH = D // H  # 64
NC = 8  # cores
NB = B // NC  # batches per core
P = 128
CH = D // P  # 4 chunks of 128
EPS = 1e-5
SCALE = 1.0 / math.sqrt(D)


def _default_cfg():
    return dict(bq_zero=True, bk_zero=True, bv_zero=True, bo_zero=True,
                aff0_triv=True, aff1_triv=True)


def _build_program(cfg):
    """Builds the SPMD Bass program. cfg holds specialization flags."""
    import concourse.bass as bass
    import concourse.mybir as mybir
    import concourse.tile as tile
    from concourse import bacc
    from concourse.masks import make_identity

    f32 = mybir.dt.float32
    bf16 = mybir.dt.bfloat16
    fp8 = mybir.dt.float8e4
    AF = mybir.ActivationFunctionType
    OP = mybir.AluOpType
    DR = mybir.MatmulPerfMode.DoubleRow

    nc = bacc.Bacc("TRN2")

    # ---- DRAM tensors (per-core shard) ----
    QT = nc.dram_tensor("QT", [NB, D, S], bf16, kind="ExternalInput")
    KT = nc.dram_tensor("KT", [NB, D, S], bf16, kind="ExternalInput")
    VT = nc.dram_tensor("VT", [NB, D, S], bf16, kind="ExternalInput")
    WQT = nc.dram_tensor("WQT", [D, D], bf16, kind="ExternalInput")  # [d, e]
    WKT = nc.dram_tensor("WKT", [D, D], bf16, kind="ExternalInput")
    WVT = nc.dram_tensor("WVT", [D, D], bf16, kind="ExternalInput")
    WOT = nc.dram_tensor("WOT", [D, D], bf16, kind="ExternalInput")  # [e, f]
    BQ = nc.dram_tensor("BQ", [D], f32, kind="ExternalInput")
    BK = nc.dram_tensor("BK", [D], f32, kind="ExternalInput")
    OUT = nc.dram_tensor("OUT", [NB, S, D], bf16, kind="ExternalOutput")
    if not cfg["bv_zero"]:
        BV = nc.dram_tensor("BV", [D], f32, kind="ExternalInput")
    if not cfg["bo_zero"]:
        BO = nc.dram_tensor("BO", [D], f32, kind="ExternalInput")
    if not cfg["aff0_triv"]:
        G0 = nc.dram_tensor("G0", [D], f32, kind="ExternalInput")
        B0 = nc.dram_tensor("B0", [D], f32, kind="ExternalInput")
    if not cfg["aff1_triv"]:
        G1 = nc.dram_tensor("G1", [D], f32, kind="ExternalInput")
        B1 = nc.dram_tensor("B1", [D], f32, kind="ExternalInput")

    def bcast_ap(vec_ap, parts=P):
        # [D] dram vector -> [parts, D] partition-broadcast AP
        return bass.AP(
            tensor=vec_ap.tensor,
            offset=vec_ap.offset,
            ap=[[0, parts]] + list(vec_ap.ap),
        )

    with tile.TileContext(nc) as tc, ExitStack() as ctx:
        singles = ctx.enter_context(tc.tile_pool(name="singles", bufs=1))
        wpool = ctx.enter_context(tc.tile_pool(name="wpool", bufs=1))
        inp = ctx.enter_context(tc.tile_pool(name="inp", bufs=3))
        # bufs=3: batch b+1's projections must not wait for attention(b-1)
        # to release its qT/kT tiles, or the scheduler freezes MLP(b) ahead
        # of proj(b+1) in PE order and the PE drains at batch boundaries
        proj = ctx.enter_context(tc.tile_pool(name="proj", bufs=2))
        attn = ctx.enter_context(tc.tile_pool(name="attn", bufs=2))
        work = ctx.enter_context(tc.tile_pool(name="work", bufs=2))
        outz = ctx.enter_context(tc.tile_pool(name="outz", bufs=3))
        # PSUM: exactly 8 banks.  Separate tags per pipeline stage so the
        # FIFO ring of one stage never makes the next batch's projections
        # wait on this batch's MLP (the rings are the real serializers).
        ps_mm = ctx.enter_context(tc.tile_pool(name="ps_mm", bufs=2, space="PSUM"))
        ps_my = ctx.enter_context(tc.tile_pool(name="ps_my", bufs=1, space="PSUM"))
        ps_sc = ctx.enter_context(tc.tile_pool(name="ps_sc", bufs=2, space="PSUM"))
        ps_pn = ctx.enter_context(tc.tile_pool(name="ps_pn", bufs=2, space="PSUM"))
        ps_sp = ctx.enter_context(tc.tile_pool(name="ps_sp", bufs=1, space="PSUM"))

        # ---- one-time constants ----
        ident_b = singles.tile([P, P], bf16)
        make_identity(nc, ident_b)

        def newton_rsqrt(y, var_ap, tg, eng, wd=CH):
            # y <- rsqrt(var_ap + EPS); y is [P, CH, 1] fp32.  Seed
            # 2/(1+w) is within ~15% for w in [0.5, 4]; 3 Newton steps
            # land below 1e-4 relative from anywhere in [0.15, 8].
            # Mid-stream batches run on GPSIMD to keep the busy DVE out
            # of this serial chain; the last batch runs on the (then
            # idle) DVE to shorten the tail.
            w = work.tile([P, wd, 1], f32, name=f"nw{tg}", tag=f"nw{tg}")
            t = work.tile([P, wd, 1], f32, name=f"nt{tg}", tag=f"nt{tg}")
            eng.tensor_scalar_add(w, var_ap, EPS)
            eng.tensor_scalar_add(y, w, 1.0)
            nc.vector.reciprocal(y, y)
            eng.tensor_scalar_mul(y, y, 2.0)
            for _ in range(2):
                eng.tensor_mul(t, y, y)
                eng.tensor_mul(t, t, w)
                eng.tensor_scalar(
                    out=t, in0=t, scalar1=-0.5, scalar2=1.5,
                    op0=OP.mult, op1=OP.add,
                )
                eng.tensor_mul(y, y, t)

        # weights resident: one [128, 4(d-chunk), 512] tile per tensor,
        # loaded by a single batched DMA each.  q/k weights first (on the
        # SWDGE/Pool queue) so the first projections start as early as
        # possible while inputs stream in on the SP queue.
        wq = wpool.tile([P, CH, D], bf16, name="wq", tag="wq")
        wk = wpool.tile([P, CH, D], bf16, name="wk", tag="wk")
        wv = wpool.tile([P, CH, D], bf16, name="wv", tag="wv")
        wo = wpool.tile([P, CH, D], bf16, name="wo", tag="wo")
        nc.gpsimd.dma_start(wq, WQT[:, :].rearrange("(c p) e -> p c e", p=P))
        nc.gpsimd.dma_start(wk, WKT[:, :].rearrange("(c p) e -> p c e", p=P))
        nc.gpsimd.dma_start(wv, WVT[:, :].rearrange("(c p) e -> p c e", p=P))
        nc.gpsimd.dma_start(wo, WOT[:, :].rearrange("(c p) e -> p c e", p=P))

        # biases for qT/kT drains: [128, 4] (partition = e % 128, col = e // 128)
        bq_sb = singles.tile([P, CH], f32)
        nc.sync.dma_start(bq_sb, BQ[:].rearrange("(c p) -> p c", p=P))
        bk_sb = singles.tile([P, CH], f32)
        nc.sync.dma_start(bk_sb, BK[:].rearrange("(c p) -> p c", p=P))
        if not cfg["bq_zero"]:
            bq_nat_b = singles.tile([P, D], f32)
            nc.sync.dma_start(bq_nat_b, bcast_ap(BQ[:]))
        if not cfg["bv_zero"]:
            bv_b = singles.tile([P, D], f32)
            nc.sync.dma_start(bv_b, bcast_ap(BV[:]))
        if not cfg["bo_zero"]:
            bo_b = singles.tile([P, D], f32)
            nc.sync.dma_start(bo_b, bcast_ap(BO[:]))
        if not cfg["aff0_triv"]:
            g0_b = singles.tile([P, D], f32)
            nc.sync.dma_start(g0_b, bcast_ap(G0[:]))
            b0_b = singles.tile([P, D], f32)
            nc.sync.dma_start(b0_b, bcast_ap(B0[:]))
        if not cfg["aff1_triv"]:
            g1_b = singles.tile([P, D], f32)
            nc.sync.dma_start(g1_b, bcast_ap(G1[:]))
            b1_b = singles.tile([P, D], f32)
            nc.sync.dma_start(b1_b, bcast_ap(B1[:]))

        def stage_head(b):
            """Input loads + projections + q_nat for batch b."""
            st = {}
            # one batched DMA per tensor (pre-transposed [d, seq] bf16)
            qt_in = inp.tile([P, CH, S], bf16, name="qt", tag="qt")
            kt_in = inp.tile([P, CH, S], bf16, name="kt", tag="kt")
            vt_in = inp.tile([P, CH, S], bf16, name="vt", tag="vt")
            nc.sync.dma_start(qt_in, QT[b].rearrange("(c p) s -> p c s", p=P))
            nc.sync.dma_start(kt_in, KT[b].rearrange("(c p) s -> p c s", p=P))
            nc.sync.dma_start(vt_in, VT[b].rearrange("(c p) s -> p c s", p=P))

            # qT/kT: [e-chunk 128, i 512]; drain with per-partition bias add
            qT = [proj.tile([P, S], bf16, name=f"qT{c}", tag=f"qT{c}") for c in range(CH)]
            kT = [proj.tile([P, S], bf16, name=f"kT{c}", tag=f"kT{c}") for c in range(CH)]
            for c in range(CH):
                ps = ps_mm.tile([P, S], f32, name="mm", tag="mm")
                for dc in range(CH):
                    nc.tensor.matmul(
                        ps, lhsT=wq[:, dc, c * P : (c + 1) * P], rhs=qt_in[:, dc, :],
                        start=(dc == 0), stop=(dc == CH - 1),
                    )
                if cfg["bq_zero"]:
                    nc.vector.tensor_copy(qT[c], ps)
                else:
                    nc.vector.tensor_scalar_add(qT[c], ps, bq_sb[:, c : c + 1])
                ps = ps_mm.tile([P, S], f32, name="mm", tag="mm")
                for dc in range(CH):
                    nc.tensor.matmul(
                        ps, lhsT=wk[:, dc, c * P : (c + 1) * P], rhs=kt_in[:, dc, :],
                        start=(dc == 0), stop=(dc == CH - 1),
                    )
                if cfg["bk_zero"]:
                    # k drains on DVE to balance ACT (which owns exp + q)
                    nc.vector.tensor_copy(kT[c], ps)
                else:
                    nc.vector.tensor_scalar_add(kT[c], ps, bk_sb[:, c : c + 1])

            # v natural [j-chunk 128, head, 65] with ones column per head,
            # fp8 with the two j-chunks of a k-tile pair stacked on dim 1
            # (DoubleRow moving-operand layout)
            v8 = [proj.tile([P, 2, H, DH + 1], fp8, name=f"va{t}", tag=f"va{t}") for t in range(2)]
            for c in range(CH):
                ps = ps_mm.tile([P, S], f32, name="mm", tag="mm")
                for dc in range(CH):
                    nc.tensor.matmul(
                        ps, lhsT=vt_in[:, dc, c * P : (c + 1) * P], rhs=wv[:, dc, :],
                        start=(dc == 0), stop=(dc == CH - 1),
                    )
                psv = ps[:].rearrange("p (h d) -> p h d", h=H)
                dst = v8[c // 2][:, c % 2, :, :]
                if cfg["bv_zero"]:
                    nc.vector.tensor_copy(dst[:, :, 0:DH], psv)
                else:
                    bvv = bv_b[:].rearrange("p (h d) -> p h d", h=H)
                    nc.vector.scalar_tensor_tensor(
                        out=dst[:, :, 0:DH], in0=psv, scalar=0.0, in1=bvv,
                        op0=OP.add, op1=OP.add,
                    )
                nc.gpsimd.memset(dst[:, :, DH : DH + 1], 1.0)

            # q natural for the residual via DMA-XBAR transpose of qT:
            # one tile [128, 4(i-chunk), 512(e)]; qn[p, c, e] = q[c*128+p, e].
            # Costs no PE or DVE time at all.
            qn = work.tile([P, CH, S], bf16, name="qn", tag="qn")
            for ec in range(CH):
                nc.sync.dma_start_transpose(
                    qn[:, :, ec * P : (ec + 1) * P], qT[ec][:, :])
            if cfg["bq_zero"]:
                q_nat = [qn[:, c, :] for c in range(CH)]
            else:
                # bias varies along the free dim; broadcast add per chunk
                qb = work.tile([P, CH, S], bf16, name="qb", tag="qb")
                for c in range(CH):
                    nc.vector.scalar_tensor_tensor(
                        out=qb[:, c, :], in0=qn[:, c, :], scalar=0.0,
                        in1=bq_nat_b, op0=OP.add, op1=OP.add,
                    )
                q_nat = [qb[:, c, :] for c in range(CH)]
            st.update(qT=qT, kT=kT, v8=v8, q_nat=q_nat)
            return st

        def stage_attn(b, st):
            """Attention for batch b: scoresT -> exp -> fp8-DR AV -> x0."""
            qT, kT, v8, q_nat = st["qT"], st["kT"], st["v8"], st["q_nat"]
            x0 = [work.tile([P, S], bf16, name=f"x0{c}", tag=f"x0{c}") for c in range(CH)]
            # softmax denominators for the whole batch: [i, ic, h] (1 bank)
            sps = ps_sp.tile([P, CH, H], f32, name="sps", tag="sps")
            for hp in range(H // 2):
                h0, h1 = 2 * hp, 2 * hp + 1
                ec = hp
                # pt in fp8, j-chunk pairs stacked on dim 1 for DoubleRow
                pt = [attn.tile([P, 2, 2, S], fp8, name=f"pt{t}", tag=f"pt{t}")
                      for t in range(2)]
                for jc in range(CH):
                    for idx, h in enumerate((h0, h1)):
                        r0 = (h % 2) * DH
                        ssc = ps_sc.tile([P, S], f32, name="sc", tag="sc")
                        nc.tensor.matmul(
                            ssc,
                            lhsT=kT[ec][r0 : r0 + DH, jc * P : (jc + 1) * P],
                            rhs=qT[ec][r0 : r0 + DH, :],
                            start=True, stop=True,
                        )
                        nc.scalar.activation(
                            pt[jc // 2][:, jc % 2, idx, :], ssc, AF.Exp, scale=SCALE)
                # AV natural, fp8 DoubleRow: each matmul contracts two
                # 128-row j-chunks at half a cycle per output row.  The
                # whole pair fits one PSUM bank ([128, 4, 2, 64] = 2048B);
                # denominators come from N=1 matmuls against the v8 ones
                # column (same stationary pt, so the weight reload is free)
                pn = ps_pn.tile([P, CH, 2, DH], f32, name="pn", tag="pn")
                for ic in range(CH):
                    for idx, h in enumerate((h0, h1)):
                        for t in range(2):
                            nc.tensor.matmul(
                                pn[:, ic, idx, :],
                                lhsT=pt[t][:, :, idx, ic * P : (ic + 1) * P],
                                rhs=v8[t][:, :, h, 0:DH],
                                start=(t == 0), stop=(t == 1),
                                perf_mode=DR,
                            )
                        for t in range(2):
                            nc.tensor.matmul(
                                sps[:, ic, h : h + 1],
                                lhsT=pt[t][:, :, idx, ic * P : (ic + 1) * P],
                                rhs=v8[t][:, :, h, DH : DH + 1],
                                start=(t == 0), stop=(t == 1),
                                perf_mode=DR,
                            )
                # x0: one reciprocal per pair, then per-head fused
                # (o * 1/s) + q_nat on DVE (GPSIMD cannot read PSUM)
                rec = work.tile([P, CH, 2], f32, name=f"rc{hp % 2}", tag=f"rc{hp % 2}")
                nc.vector.reciprocal(rec, sps[:, :, h0 : h0 + 2])
                for ic in range(CH):
                    for idx, h in enumerate((h0, h1)):
                        nc.vector.scalar_tensor_tensor(
                            out=x0[ic][:, h * DH : (h + 1) * DH],
                            in0=pn[:, ic, idx, :],
                            scalar=rec[:, ic, idx : idx + 1],
                            in1=q_nat[ic][:, h * DH : (h + 1) * DH],
                            op0=OP.mult, op1=OP.add,
                        )
            st["x0"] = x0

        def stage_tail(b, st):
            """LN0, MLP, LN1, store for batch b."""
            x0 = st["x0"]
            st6 = work.tile([P, CH, 6], f32, name="st6a", tag="st6a")
            mv0 = work.tile([P, CH, 2], f32, name="mv0", tag="mv0")
            for ic in range(CH):
                nc.vector.bn_stats(st6[:, ic, :], x0[ic])
                nc.vector.bn_aggr(mv0[:, ic, :], st6[:, ic, :])
            late = b == NB - 1  # tail: other engines are idle, use them
            lv = nc.vector if late else nc.gpsimd
            # n0 = (x0 - mu) * rsig  (bf16 for the MLP matmul)
            n0 = [work.tile([P, S], bf16, name=f"n0{c}", tag=f"n0{c}") for c in range(CH)]

            def emit_n0(ics, rsig0, r0):
                for ic in ics:
                    nc.vector.tensor_scalar(
                        out=n0[ic], in0=x0[ic],
                        scalar1=mv0[:, ic, 0:1], scalar2=rsig0[:, ic - r0, :],
                        op0=OP.subtract, op1=OP.mult,
                    )

            if late:
                # split LN0 so the first chunks' n0T transposes and MLP
                # matmuls start while the second half still normalizes
                for hh in range(2):
                    rs = work.tile([P, 2, 1], f32, name=f"rs0{hh}", tag=f"rs0{hh}")
                    newton_rsqrt(rs, mv0[:, 2 * hh : 2 * hh + 2, 1:2], f"a{hh}", lv, wd=2)
                    emit_n0((2 * hh, 2 * hh + 1), rs, 2 * hh)
            else:
                rsig0 = work.tile([P, CH, 1], f32, name="rsig0", tag="rsig0")
                newton_rsqrt(rsig0, mv0[:, :, 1:2], "a", lv)
                emit_n0(range(CH), rsig0, 0)
            if not cfg["aff0_triv"]:
                # X = n0 * g0 + b0 (residual/LN1 path; g0 already folded in WOT)
                xr = [work.tile([P, S], f32, name=f"xr{c}", tag=f"xr{c}") for c in range(CH)]
                for ic in range(CH):
                    nc.vector.tensor_tensor(xr[ic], n0[ic], g0_b, op=OP.mult)
                    nc.vector.tensor_tensor(xr[ic], xr[ic], b0_b, op=OP.add)
            else:
                xr = n0

            # ---- MLP: n0T, Y = n0 @ Wo'.T, z = X + relu(Y + bo), LN1 ----
            # n0T via DMA-XBAR transpose: nt[p, c, i] = n0T[c*128+p, i].
            # Issued on the ACT queue so input prefetches on the SP queue
            # are never stuck behind it.
            n0T = work.tile([P, CH, S], bf16, name="nt", tag="nt")
            for ic in range(CH):
                (nc.sync if late else nc.scalar).dma_start_transpose(
                    n0T[:, :, ic * P : (ic + 1) * P], n0[ic][:, :])

            z = [outz.tile([P, S], bf16, name=f"z{c}", tag=f"z{c}") for c in range(CH)]
            st6b = work.tile([P, CH, 6], f32, name="st6b", tag="st6b")
            mv1 = work.tile([P, CH, 2], f32, name="mv1", tag="mv1")
            for ic in range(CH):
                # the last batch has no successor competing for the proj
                # ring, so borrow it to double-buffer the final MLP
                yps = (ps_mm.tile([P, S], f32, name="mm", tag="mm") if late
                       else ps_my.tile([P, S], f32, name="my", tag="my"))
                for ec in range(CH):
                    nc.tensor.matmul(
                        yps, lhsT=n0T[:, ec, ic * P : (ic + 1) * P], rhs=wo[:, ec, :],
                        start=(ec == 0), stop=(ec == CH - 1),
                    )
                if not cfg["bo_zero"]:
                    nc.vector.tensor_tensor(yps, yps, bo_b, op=OP.add)
                # drain+relu on ACT, residual add on DVE at bf16 2x rate:
                # keeps the top-loaded DVE out of the MLP drain path
                ry = work.tile([P, S], bf16, name=f"ry{ic % 2}", tag=f"ry{ic % 2}")
                nc.scalar.activation(ry, yps, AF.Relu)
                nc.vector.tensor_tensor(z[ic], ry, xr[ic], op=OP.add)
                nc.vector.bn_stats(st6b[:, ic, :], z[ic])
                nc.vector.bn_aggr(mv1[:, ic, :], st6b[:, ic, :])

            def emit_out(ics, rsig1, r0):
                for ic in ics:
                    o_sb = outz.tile([P, S], bf16, name=f"os{ic % 2}", tag=f"os{ic % 2}")
                    nc.vector.tensor_scalar(
                        out=o_sb, in0=z[ic],
                        scalar1=mv1[:, ic, 0:1], scalar2=rsig1[:, ic - r0, :],
                        op0=OP.subtract, op1=OP.mult,
                    )
                    if not cfg["aff1_triv"]:
                        nc.vector.tensor_tensor(o_sb, o_sb, g1_b, op=OP.mult)
                        nc.vector.tensor_tensor(o_sb, o_sb, b1_b, op=OP.add)
                    # outputs go out on the SWDGE (Pool) queue so they never
                    # head-of-line block the input loads on the SP queue; the
                    # last batch uses the (now idle) SP/HWDGE queue instead
                    (nc.sync if late else nc.gpsimd).dma_start(
                        OUT[b, ic * P : (ic + 1) * P, :], o_sb)

            if late:
                # split LN1 so the first half's outputs leave while the
                # second half's MLP is still draining
                for hh in range(2):
                    rs = work.tile([P, 2, 1], f32, name=f"rs1{hh}", tag=f"rs1{hh}")
                    newton_rsqrt(rs, mv1[:, 2 * hh : 2 * hh + 2, 1:2], f"b{hh}", lv, wd=2)
                    emit_out((2 * hh, 2 * hh + 1), rs, 2 * hh)
            else:
                rsig1 = work.tile([P, CH, 1], f32, name="rsig1", tag="rsig1")
                newton_rsqrt(rsig1, mv1[:, :, 1:2], "b", lv)
                emit_out(range(CH), rsig1, 0)

        # Sequential emission per batch.  (Both priority-based and
        # emission-based cross-batch reordering produce schedules that
        # hang real hardware, so the stages stay in order; the Tile
        # scheduler still overlaps across batches where dependencies and
        # pool rings allow.)
        for b in range(NB):
            st = stage_head(b)
            stage_attn(b, st)
            stage_tail(b, st)

    nc.finalize()
    return nc


def kernel(**inputs) -> np.ndarray:
    import ml_dtypes

    from concourse.bass_utils import run_bass_kernel_spmd

    f32 = np.float32
    bf16 = ml_dtypes.bfloat16
    Q = np.asarray(inputs["Q"], dtype=f32)
    K = np.asarray(inputs["K"], dtype=f32)
    V = np.asarray(inputs["V"], dtype=f32)
    Wq = np.asarray(inputs["Wq"], dtype=f32)
    Wk = np.asarray(inputs["Wk"], dtype=f32)
    Wv = np.asarray(inputs["Wv"], dtype=f32)
    Wo = np.asarray(inputs["Wo"], dtype=f32)
    bq = np.asarray(inputs["bq"], dtype=f32)
    bk = np.asarray(inputs["bk"], dtype=f32)
    bv = np.asarray(inputs["bv"], dtype=f32)
    bo = np.asarray(inputs["bo"], dtype=f32)
    g0 = np.asarray(inputs["g0"], dtype=f32)
    b0 = np.asarray(inputs["b0"], dtype=f32)
    g1 = np.asarray(inputs["g1"], dtype=f32)
    b1 = np.asarray(inputs["b1"], dtype=f32)

    cfg = {
        "bq_zero": not np.any(bq),
        "bk_zero": not np.any(bk),
        "bv_zero": not np.any(bv),
        "bo_zero": not np.any(bo),
        "aff0_triv": bool(np.all(g0 == 1.0) and not np.any(b0)),
        "aff1_triv": bool(np.all(g1 == 1.0) and not np.any(b1)),
    }

    # Fold g0 into Wo (valid in general: X@Wo.T = (n0*g0+b0)@Wo.T uses
    # Wo' = Wo * g0 on the input axis; the b0 term folds into bo).
    Wo_f = Wo * g0[None, :]
    bo_f = bo + Wo @ b0

    cfg["bo_zero"] = not np.any(bo_f)

    nc = _build_program(cfg)

    in_maps = []
    for c in range(NC):
        sl = slice(c * NB, (c + 1) * NB)
        m = {
            "QT": np.ascontiguousarray(Q[sl].transpose(0, 2, 1)).astype(bf16),
            "KT": np.ascontiguousarray(K[sl].transpose(0, 2, 1)).astype(bf16),
            "VT": np.ascontiguousarray(V[sl].transpose(0, 2, 1)).astype(bf16),
            "WQT": np.ascontiguousarray(Wq.T).astype(bf16),
            "WKT": np.ascontiguousarray(Wk.T).astype(bf16),
            "WVT": np.ascontiguousarray(Wv.T).astype(bf16),
            "WOT": np.ascontiguousarray(Wo_f.T).astype(bf16),
            "BQ": bq,
            "BK": bk,
        }
        if not cfg["bv_zero"]:
            m["BV"] = bv
        if not cfg["bo_zero"]:
            m["BO"] = bo_f
        if not cfg["aff0_triv"]:
            m["G0"] = g0
            m["B0"] = b0
        if not cfg["aff1_triv"]:
            m["G1"] = g1
            m["B1"] = b1
        in_maps.append(m)

    res = run_bass_kernel_spmd(nc, in_maps, core_ids=list(range(NC)))
    out = np.concatenate([r["OUT"] for r in res.results], axis=0)
    return out.astype(np.float32)


if __name__ == "__main__":
    rng = np.random.default_rng(0)
    ins = {
        "Q": rng.standard_normal((B, S, D), dtype=np.float32),
        "K": rng.standard_normal((B, S, D), dtype=np.float32),
        "V": rng.standard_normal((B, S, D), dtype=np.float32),
        "Wq": rng.standard_normal((D, D), dtype=np.float32) / math.sqrt(D),
        "bq": np.zeros(D, np.float32),
        "Wk": rng.standard_normal((D, D), dtype=np.float32) / math.sqrt(D),
        "bk": np.zeros(D, np.float32),
        "Wv": rng.standard_normal((D, D), dtype=np.float32) / math.sqrt(D),
        "bv": np.zeros(D, np.float32),
        "Wo": rng.standard_normal((D, D), dtype=np.float32) / math.sqrt(D),
        "bo": np.zeros(D, np.float32),
        "g0": np.ones(D, np.float32),
        "b0": np.zeros(D, np.float32),
        "g1": np.ones(D, np.float32),
        "b1": np.zeros(D, np.float32),
    }
    out = kernel(**ins)
    print(out.shape, out.dtype)
